# revision 27
# baseline (speedup 1.0000x reference)
"""Multi-head attention (B=2, S=2048, D=768, H=12) on 8 Trainium2 cores.

Sharding: core c -> batch b = c // 4, head-group g = c % 4 (3 heads of 12).
Host prep: x^T per batch pre-transposed AND cast to bf16 (halves the x DMA
vs fp32+casting-DMA); weight shards cast to bf16.  Each core projects
Q/K/V for its 3 heads, runs attention, emits its Wo row-shard partial as
bf16; the host sums 4 partials per batch in fp32 and adds bo.

Device kernel (per core):
  - Q^T/K^T stored zero-PADDED to 128 contraction rows ([128, 3, S] tiles,
    rows 64-127 = 0) so every scores matmul is a full 128x128-array
    instruction: the 64-row (half-array / HAM k=4) config measured ~2x
    slower sustained on HW (activity throttle), and padding costs no extra
    PE cycles (row count = rhs free size).
  - All matmul outputs are <=512 fp32 columns (one PSUM bank; 1024-col out
    is an ISA violation, probed).
  - Attention runs as 6 phases (qh-major: (h0,h1,h2) x qh0 then qh1), each
    16 kt steps of: scores 2mm -> exp (ScalarE, [128,1024] tiles) -> PV
    2mm accumulating [65,1024] (ones column in V_aug rides the softmax
    denominator).  A global 2-step software pipeline (scores of step i+2
    emitted before PV of step i) keeps the in-order PE queue from ever
    waiting on the ACT exp, across phase boundaries too.
  - Wo tiles for the first q-half are emitted right after (h2,qh0)'s
    normalize, shortening the serial tail to normalize + 8 wo tiles.
"""

import sys

for _p in ("/opt/trn_rl_repo",):
    if _p not in sys.path:
        sys.path.append(_p)

import numpy as np

B = 2
S = 2048
D = 768
H = 12
DK = 64
HG = 3            # heads per core
HD = HG * DK      # 192
P = 128
NS = S // P       # 16 k-tiles
ND = D // P       # 6 d-chunks
QH = 1024         # q half

_CACHE = {}


def _build_nc(use_bias_qkv):
    import concourse.bacc as bacc
    import concourse.tile as tile
    from concourse import mybir
    from contextlib import ExitStack

    BF = mybir.dt.bfloat16
    F32 = mybir.dt.float32
    EXP = mybir.ActivationFunctionType.Exp

    nc = bacc.Bacc("TRN2", target_bir_lowering=False, debug=False)

    xqT = nc.dram_tensor("xqT", [D, S], BF, kind="ExternalInput").ap()
    xkT = nc.dram_tensor("xkT", [D, S], BF, kind="ExternalInput").ap()
    xvT = nc.dram_tensor("xvT", [D, S], BF, kind="ExternalInput").ap()
    wq = nc.dram_tensor("wq", [D, HD], BF, kind="ExternalInput").ap()
    wk = nc.dram_tensor("wk", [D, HD], BF, kind="ExternalInput").ap()
    wv = nc.dram_tensor("wv", [D, HD], BF, kind="ExternalInput").ap()
    wo = nc.dram_tensor("wo", [HD, D], BF, kind="ExternalInput").ap()
    bqkv = nc.dram_tensor("bqkv", [3, HD], F32, kind="ExternalInput").ap()
    y = nc.dram_tensor("y", [S, D], BF, kind="ExternalOutput").ap()

    with tile.TileContext(nc) as tc, ExitStack() as ctx:
        wpool = ctx.enter_context(tc.tile_pool(name="weights", bufs=1))
        apool = ctx.enter_context(tc.tile_pool(name="acts", bufs=1))

        # zero-padded transposed activations: [:, h, :] = head h, rows 64+ = 0
        KT = apool.tile([P, HG, S], BF, tag="kt")
        QT = apool.tile([P, HG, S], BF, tag="qt")
        V = apool.tile([P, NS, 3 * 65], BF, tag="v")
        OC1 = apool.tile([P, S], BF, tag="oc1")    # heads 0,1 of O^T (normalized)
        OC2 = apool.tile([P, S], BF, tag="oc2")    # head 2, rows 64-127 = 0 (keeps
                                                   # the wo matmuls in full-array config)

        # x chunk tiles (bf16 straight from HBM), all resident
        # x chunks DMA'd in s-halves, first halves of all d-chunks first, so
        # the sbp0 projections can start ~4us after the tensor's DMA begins
        xt_pool = ctx.enter_context(tc.tile_pool(name="xt", bufs=1))
        xtc = {}
        for name, xT in (("wk", xkT), ("wq", xqT), ("wv", xvT)):
            for dc in range(ND):
                xtc[(name, dc)] = xt_pool.tile(
                    [P, S], BF, tag=f"xt_{name}{dc}", name=f"xt_{name}{dc}"
                )
            for half in range(2):
                hsl = slice(half * QH, (half + 1) * QH)
                for dc in range(ND):
                    # alternate DMA queues: one queue alone doesn't saturate
                    # HBM read bandwidth, and x load paces the whole prologue
                    q = nc.gpsimd if dc % 2 == 0 else nc.sync
                    q.dma_start(
                        out=xtc[(name, dc)][:, hsl],
                        in_=xT[dc * P : (dc + 1) * P, hsl],
                    )

        # weights (bf16 on host, no device casts), HWDGE queue
        w_bf = {}
        for name, w in (("wk", wk), ("wq", wq), ("wv", wv)):
            wb = wpool.tile([P, ND, HD], BF, tag=f"{name}_bf", name=f"{name}_bf")
            nc.sync.dma_start(out=wb, in_=w.rearrange("(nd p) h -> p nd h", p=P))
            w_bf[name] = wb
        wo_b1 = wpool.tile([P, D], BF, tag="wo_b1")
        nc.sync.dma_start(out=wo_b1, in_=wo[0:P, :])
        wo_b2 = wpool.tile([P, D], BF, tag="wo_b2")   # rows 64-127 = 0 (padding)
        nc.sync.dma_start(out=wo_b2[0:DK, :], in_=wo[P:HD, :])

        bias_a = {}
        bias_b = {}
        bias_vrow = None
        if use_bias_qkv:
            for i, name in enumerate(("wq", "wk", "wv")):
                ba = wpool.tile([P, 1], F32, tag=f"ba_{name}", name=f"ba_{name}")
                nc.sync.dma_start(out=ba, in_=bqkv[i, 0:P].rearrange("p -> p 1"))
                bb = wpool.tile([DK, 1], F32, tag=f"bb_{name}", name=f"bb_{name}")
                nc.sync.dma_start(out=bb, in_=bqkv[i, P:HD].rearrange("p -> p 1"))
                bias_a[name] = ba
                bias_b[name] = bb
            # V bias varies along the free dim of psV [s, 192]: broadcast the
            # bias row across all 128 partitions once
            bvr = wpool.tile([1, HD], F32, tag="bv_row")
            nc.sync.dma_start(out=bvr, in_=bqkv[2, :].rearrange("h -> 1 h"))
            bias_vrow = wpool.tile([P, HD], F32, tag="bv_bcast")
            nc.gpsimd.partition_broadcast(bias_vrow, bvr)

        # padding zeros + V ones columns (off the PE path; after DMA triggers)
        nc.gpsimd.memset(KT[DK:P, :, :], 0.0)
        nc.vector.memset(QT[DK:P, :, :], 0.0)
        nc.vector.memset(V[:, :, 64 : 3 * 65 : 65], 1.0)
        nc.gpsimd.memset(OC2[DK:P, :], 0.0)
        nc.vector.memset(wo_b2[DK:P, :], 0.0)

        # ================= phase 1: projections =================
        with tc.tile_pool(name="ppa", bufs=2, space="PSUM") as ppa_pool, \
             tc.tile_pool(name="ppb", bufs=1, space="PSUM") as ppb_pool, \
             tc.tile_pool(name="psv", bufs=2, space="PSUM") as psv_pool:

            def qk_proj(name, dst):
                wb = w_bf[name]
                for sbp in range(2):
                    sl = slice(sbp * QH, (sbp + 1) * QH)
                    psA = ppa_pool.tile([P, QH], F32, tag="ppa", name=f"psA_{name}{sbp}")
                    psB = ppb_pool.tile([DK, QH], F32, tag="ppb", name=f"psB_{name}{sbp}")
                    for d in range(ND):
                        xt_d = xtc[(name, d)]
                        for half in range(2):
                            hsl = slice(half * 512, (half + 1) * 512)
                            xsl = slice(sbp * QH + half * 512, sbp * QH + (half + 1) * 512)
                            nc.tensor.matmul(
                                psA[:, hsl], wb[:, d, 0:P], xt_d[:, xsl],
                                start=(d == 0), stop=(d == ND - 1),
                            )
                    for d in range(ND):
                        xt_d = xtc[(name, d)]
                        for half in range(2):
                            hsl = slice(half * 512, (half + 1) * 512)
                            xsl = slice(sbp * QH + half * 512, sbp * QH + (half + 1) * 512)
                            nc.tensor.matmul(
                                psB[:, hsl], wb[:, d, P:HD], xt_d[:, xsl],
                                start=(d == 0), stop=(d == ND - 1),
                            )
                    if use_bias_qkv:
                        ba, bb = bias_a[name], bias_b[name]
                        nc.vector.tensor_scalar_add(dst[0:DK, 0, sl], psA[0:DK, :], ba[0:DK])
                        nc.vector.tensor_scalar_add(dst[0:DK, 1, sl], psA[DK:P, :], ba[DK:P])
                        nc.vector.tensor_scalar_add(dst[0:DK, 2, sl], psB, bb)
                    else:
                        nc.vector.tensor_copy(out=dst[0:DK, 0, sl], in_=psA[0:DK, :])
                        nc.vector.tensor_copy(out=dst[0:DK, 1, sl], in_=psA[DK:P, :])
                        nc.vector.tensor_copy(out=dst[0:DK, 2, sl], in_=psB)

            qk_proj("wk", KT)
            qk_proj("wq", QT)

            wb = w_bf["wv"]
            for st in range(NS):
                psV = psv_pool.tile([P, HD], F32, tag="psv", name=f"psV{st}")
                for d in range(ND):
                    nc.tensor.matmul(
                        psV, xtc[("wv", d)][:, st * P : (st + 1) * P], wb[:, d, :],
                        start=(d == 0), stop=(d == ND - 1),
                    )
                for h in range(HG):
                    if use_bias_qkv:
                        nc.vector.tensor_add(
                            V[:, st, h * 65 : h * 65 + 64],
                            psV[:, h * DK : (h + 1) * DK],
                            bias_vrow[:, h * DK : (h + 1) * DK],
                        )
                    else:
                        nc.vector.tensor_copy(
                            out=V[:, st, h * 65 : h * 65 + 64],
                            in_=psV[:, h * DK : (h + 1) * DK],
                        )

        # ============ phase 2: attention (+ wo) ============
        with tc.tile_pool(name="s_ps", bufs=2, space="PSUM") as s_pool, \
             tc.tile_pool(name="ot_ps", bufs=2, space="PSUM") as ot_pool, \
             tc.tile_pool(name="pt", bufs=6) as pt_pool, \
             tc.tile_pool(name="nrm", bufs=2) as nrm_pool, \
             tc.tile_pool(name="y_sb", bufs=2) as ysb_pool:

            # qh-major so wo for q-half 0 can run before the final phase
            phases = [(h, 0) for h in range(HG)] + [(h, 1) for h in range(HG)]
            ots = {}
            pts = {}

            def s_exp(i):
                h, qh = phases[i // NS]
                kt = i % NS
                s_ps = s_pool.tile([P, QH], F32, tag="s", name=f"s{i}")
                for n in range(2):
                    q0 = qh * QH + n * 512
                    nc.tensor.matmul(
                        s_ps[:, n * 512 : (n + 1) * 512],
                        KT[:, h, kt * P : (kt + 1) * P],
                        QT[:, h, q0 : q0 + 512],
                        start=True, stop=True,
                    )
                pt = pt_pool.tile([P, QH], BF, tag="pt", name=f"pt{i}")
                nc.scalar.activation(pt, s_ps, EXP, bias=0.0, scale=0.125)
                pts[i] = pt

            def pv(i):
                h, qh = phases[i // NS]
                kt = i % NS
                if kt == 0:
                    ots[(h, qh)] = ot_pool.tile([65, QH], F32, tag="ot", name=f"ot{h}_{qh}")
                ot = ots[(h, qh)]
                pt = pts.pop(i)
                for n in range(2):
                    nc.tensor.matmul(
                        ot[:, n * 512 : (n + 1) * 512],
                        V[:, kt, h * 65 : (h + 1) * 65],
                        pt[:, n * 512 : (n + 1) * 512],
                        start=(kt == 0), stop=(kt == NS - 1),
                    )

            def normalize(h, qh):
                ot = ots.pop((h, qh))
                osb = nrm_pool.tile([DK, QH], F32, tag="osb", name="osb")
                nc.vector.tensor_copy(out=osb, in_=ot[0:DK, :])
                den = nrm_pool.tile([1, QH], F32, tag="den", name="den")
                nc.scalar.copy(den, ot[64:65, :])
                recip = nrm_pool.tile([1, QH], F32, tag="recip", name="recip")
                nc.vector.reciprocal_approx_fast(recip, den)
                rbc = nrm_pool.tile([DK, QH], F32, tag="rbc", name="rbc")
                nc.gpsimd.partition_broadcast(rbc, recip)
                sl = slice(qh * QH, (qh + 1) * QH)
                dst = OC1[0:DK, sl] if h == 0 else (OC1[DK:P, sl] if h == 1 else OC2[0:DK, sl])
                nc.vector.tensor_mul(dst, osb, rbc)

            y_r = y.rearrange("(n p) m -> n p m", p=P)

            def writeback(dst, y_ps, st, tail=False):
                y_sb = ysb_pool.tile([P, D], BF, tag="ysb", name=f"y_sb{st}")
                nc.vector.tensor_copy(out=y_sb, in_=y_ps)
                nc.sync.dma_start(out=dst, in_=y_sb)

            def wo_parts(y_ps, st, parts):
                sl = slice(st * P, (st + 1) * P)
                # region-major order: adjacent matmuls never accumulate into
                # the same PSUM region (back-to-back same-bank accumulation
                # serializes the PE)
                last = len(parts) - 1
                for pi, (oc, wob) in enumerate(parts):
                    for n0, nn in ((0, 512), (512, 256)):
                        nc.tensor.matmul(
                            y_ps[:, n0 : n0 + nn], oc[:, sl], wob[:, n0 : n0 + nn],
                            start=(pi == 0), stop=(pi == last),
                        )

            def wo_tile(st, tail=False):
                y_ps = ot_pool.tile([P, D], F32, tag="ot", name=f"y_ps{st}")
                wo_parts(y_ps, st, [(OC1, wo_b1), (OC2, wo_b2)])
                writeback(y_r[st], y_ps, st, tail=tail)

            n_steps = len(phases) * NS
            s_exp(0)
            s_exp(1)
            wo_pending = []
            wo_emitted = 0
            for i in range(n_steps):
                if i + 2 < n_steps:
                    s_exp(i + 2)
                pv(i)
                if wo_pending:
                    # spread q-half-0 wo tiles one per step instead of a
                    # blocking burst
                    wo_tile(wo_pending.pop(0))
                    wo_emitted += 1
                h, qh = phases[i // NS]
                kt = i % NS
                if kt == NS - 1:
                    normalize(h, qh)
                    if (h, qh) == (HG - 1, 0):
                        wo_pending = list(range(NS // 2))
            for st in range(wo_emitted, NS):
                wo_tile(st, tail=True)

    nc.compile()
    return nc


def kernel(query, key, value, Wq, bq, Wk, bk, Wv, bv, Wo, bo, **_ignored):
    import ml_dtypes
    from concourse.bass_utils import run_bass_kernel_spmd

    bf16 = ml_dtypes.bfloat16
    query = np.asarray(query, dtype=np.float32)
    key = np.asarray(key, dtype=np.float32)
    value = np.asarray(value, dtype=np.float32)
    Wq = np.asarray(Wq, dtype=np.float32)
    Wk = np.asarray(Wk, dtype=np.float32)
    Wv = np.asarray(Wv, dtype=np.float32)
    Wo = np.asarray(Wo, dtype=np.float32)
    bq = np.asarray(bq, dtype=np.float32)
    bk = np.asarray(bk, dtype=np.float32)
    bv = np.asarray(bv, dtype=np.float32)
    bo = np.asarray(bo, dtype=np.float32)

    use_bias_qkv = bool(np.any(bq) or np.any(bk) or np.any(bv))
    if "nc" not in _CACHE or _CACHE.get("bias") != use_bias_qkv:
        _CACHE["nc"] = _build_nc(use_bias_qkv)
        _CACHE["bias"] = use_bias_qkv
    nc = _CACHE["nc"]

    xT = {b: {} for b in range(B)}
    for b in range(B):
        xT[b]["q"] = np.ascontiguousarray(query[b].T).astype(bf16)
        xT[b]["k"] = np.ascontiguousarray(key[b].T).astype(bf16)
        xT[b]["v"] = np.ascontiguousarray(value[b].T).astype(bf16)

    in_maps = []
    for c in range(8):
        b, g = divmod(c, 4)
        hs = slice(g * HD, (g + 1) * HD)
        in_maps.append({
            "xqT": xT[b]["q"],
            "xkT": xT[b]["k"],
            "xvT": xT[b]["v"],
            "wq": np.ascontiguousarray(Wq[:, hs]).astype(bf16),
            "wk": np.ascontiguousarray(Wk[:, hs]).astype(bf16),
            "wv": np.ascontiguousarray(Wv[:, hs]).astype(bf16),
            "wo": np.ascontiguousarray(Wo[hs, :]).astype(bf16),
            "bqkv": np.ascontiguousarray(
                np.stack([bq[hs], bk[hs], bv[hs]]).astype(np.float32)
            ),
        })

    res = run_bass_kernel_spmd(nc, in_maps, core_ids=list(range(8)), **_CACHE.get("run_kwargs", {}))
    _CACHE["last_result"] = res

    out = np.empty((B, S, D), dtype=np.float32)
    for b in range(B):
        acc = res.results[4 * b]["y"].astype(np.float32)
        for g in range(1, 4):
            acc = acc + res.results[4 * b + g]["y"].astype(np.float32)
        out[b] = acc + bo[None, :]
    return out


# revision 29
# speedup vs baseline: 1.1130x; 1.1130x over previous
"""Multi-head attention (B=2, S=2048, D=768, H=12) on 8 Trainium2 cores.

Sharding: core c -> batch b = c // 4, head-group g = c % 4 (3 heads of 12).
Host prep: x^T per batch pre-transposed AND cast to bf16 (halves the x DMA
vs fp32+casting-DMA); weight shards cast to bf16.  Each core projects
Q/K/V for its 3 heads, runs attention, emits its Wo row-shard partial as
bf16; the host sums 4 partials per batch in fp32 and adds bo.

Device kernel (per core):
  - Q^T/K^T stored zero-PADDED to 128 contraction rows ([128, 3, S] tiles,
    rows 64-127 = 0) so every scores matmul is a full 128x128-array
    instruction: the 64-row (half-array / HAM k=4) config measured ~2x
    slower sustained on HW (activity throttle), and padding costs no extra
    PE cycles (row count = rhs free size).
  - All matmul outputs are <=512 fp32 columns (one PSUM bank; 1024-col out
    is an ISA violation, probed).
  - Attention runs as 6 phases (qh-major: (h0,h1,h2) x qh0 then qh1), each
    16 kt steps of: scores 2mm -> exp (ScalarE, [128,1024] tiles) -> PV
    2mm accumulating [65,1024] (ones column in V_aug rides the softmax
    denominator).  A global 2-step software pipeline (scores of step i+2
    emitted before PV of step i) keeps the in-order PE queue from ever
    waiting on the ACT exp, across phase boundaries too.
  - Wo tiles for the first q-half are emitted right after (h2,qh0)'s
    normalize, shortening the serial tail to normalize + 8 wo tiles.
"""

import sys

for _p in ("/opt/trn_rl_repo",):
    if _p not in sys.path:
        sys.path.append(_p)

import numpy as np

B = 2
S = 2048
D = 768
H = 12
DK = 64
HG = 3            # heads per core
HD = HG * DK      # 192
P = 128
NS = S // P       # 16 k-tiles
ND = D // P       # 6 d-chunks
QH = 1024         # q half

_CACHE = {}


def _build_nc(use_bias_qkv):
    import concourse.bacc as bacc
    import concourse.tile as tile
    from concourse import mybir
    from contextlib import ExitStack

    BF = mybir.dt.bfloat16
    F32 = mybir.dt.float32
    EXP = mybir.ActivationFunctionType.Exp

    nc = bacc.Bacc("TRN2", target_bir_lowering=False, debug=False)

    xqT = nc.dram_tensor("xqT", [D, S], BF, kind="ExternalInput").ap()
    xkT = nc.dram_tensor("xkT", [D, S], BF, kind="ExternalInput").ap()
    xvT = nc.dram_tensor("xvT", [D, S], BF, kind="ExternalInput").ap()
    wq = nc.dram_tensor("wq", [D, HD], BF, kind="ExternalInput").ap()
    wk = nc.dram_tensor("wk", [D, HD], BF, kind="ExternalInput").ap()
    wv = nc.dram_tensor("wv", [D, HD], BF, kind="ExternalInput").ap()
    wo = nc.dram_tensor("wo", [HD, D], BF, kind="ExternalInput").ap()
    bqkv = nc.dram_tensor("bqkv", [3, HD], F32, kind="ExternalInput").ap()
    y = nc.dram_tensor("y", [S, D], BF, kind="ExternalOutput").ap()

    with tile.TileContext(nc) as tc, ExitStack() as ctx:
        wpool = ctx.enter_context(tc.tile_pool(name="weights", bufs=1))
        apool = ctx.enter_context(tc.tile_pool(name="acts", bufs=1))

        # zero-padded transposed activations: [:, h, :] = head h, rows 64+ = 0
        KT = apool.tile([P, HG, S], BF, tag="kt")
        QT = apool.tile([P, HG, S], BF, tag="qt")
        V = apool.tile([P, NS, 3 * 65], BF, tag="v")
        OC1 = apool.tile([P, S], BF, tag="oc1")    # heads 0,1 of O^T (normalized)
        OC2 = apool.tile([P, S], BF, tag="oc2")    # head 2, rows 64-127 = 0 (keeps
                                                   # the wo matmuls in full-array config)

        # x chunk tiles (bf16 straight from HBM), all resident
        # x chunks DMA'd in s-halves, first halves of all d-chunks first, so
        # the sbp0 projections can start ~4us after the tensor's DMA begins
        xt_pool = ctx.enter_context(tc.tile_pool(name="xt", bufs=1))
        xtc = {}
        for name, xT in (("wk", xkT), ("wq", xqT), ("wv", xvT)):
            for dc in range(ND):
                xtc[(name, dc)] = xt_pool.tile(
                    [P, S], BF, tag=f"xt_{name}{dc}", name=f"xt_{name}{dc}"
                )
            for half in range(2):
                hsl = slice(half * QH, (half + 1) * QH)
                for dc in range(ND):
                    nc.gpsimd.dma_start(
                        out=xtc[(name, dc)][:, hsl],
                        in_=xT[dc * P : (dc + 1) * P, hsl],
                    )

        # weights (bf16 on host, no device casts), HWDGE queue
        w_bf = {}
        for name, w in (("wk", wk), ("wq", wq), ("wv", wv)):
            wb = wpool.tile([P, ND, HD], BF, tag=f"{name}_bf", name=f"{name}_bf")
            nc.sync.dma_start(out=wb, in_=w.rearrange("(nd p) h -> p nd h", p=P))
            w_bf[name] = wb
        wo_b1 = wpool.tile([P, D], BF, tag="wo_b1")
        nc.sync.dma_start(out=wo_b1, in_=wo[0:P, :])
        wo_b2 = wpool.tile([P, D], BF, tag="wo_b2")   # rows 64-127 = 0 (padding)
        nc.sync.dma_start(out=wo_b2[0:DK, :], in_=wo[P:HD, :])

        bias_a = {}
        bias_b = {}
        bias_vrow = None
        if use_bias_qkv:
            for i, name in enumerate(("wq", "wk", "wv")):
                ba = wpool.tile([P, 1], F32, tag=f"ba_{name}", name=f"ba_{name}")
                nc.sync.dma_start(out=ba, in_=bqkv[i, 0:P].rearrange("p -> p 1"))
                bb = wpool.tile([DK, 1], F32, tag=f"bb_{name}", name=f"bb_{name}")
                nc.sync.dma_start(out=bb, in_=bqkv[i, P:HD].rearrange("p -> p 1"))
                bias_a[name] = ba
                bias_b[name] = bb
            # V bias varies along the free dim of psV [s, 192]: broadcast the
            # bias row across all 128 partitions once
            bvr = wpool.tile([1, HD], F32, tag="bv_row")
            nc.sync.dma_start(out=bvr, in_=bqkv[2, :].rearrange("h -> 1 h"))
            bias_vrow = wpool.tile([P, HD], F32, tag="bv_bcast")
            nc.gpsimd.partition_broadcast(bias_vrow, bvr)

        # padding zeros + V ones columns (off the PE path; after DMA triggers)
        nc.gpsimd.memset(KT[DK:P, :, :], 0.0)
        nc.vector.memset(QT[DK:P, :, :], 0.0)
        nc.vector.memset(V[:, :, 64 : 3 * 65 : 65], 1.0)
        nc.gpsimd.memset(OC2[DK:P, :], 0.0)
        nc.vector.memset(wo_b2[DK:P, :], 0.0)

        # ================= phase 1: projections =================
        with tc.tile_pool(name="ppa", bufs=2, space="PSUM") as ppa_pool, \
             tc.tile_pool(name="ppb", bufs=1, space="PSUM") as ppb_pool, \
             tc.tile_pool(name="psv", bufs=2, space="PSUM") as psv_pool:

            def qk_proj(name, dst):
                wb = w_bf[name]
                for sbp in range(2):
                    sl = slice(sbp * QH, (sbp + 1) * QH)
                    psA = ppa_pool.tile([P, QH], F32, tag="ppa", name=f"psA_{name}{sbp}")
                    psB = ppb_pool.tile([DK, QH], F32, tag="ppb", name=f"psB_{name}{sbp}")
                    for d in range(ND):
                        xt_d = xtc[(name, d)]
                        for half in range(2):
                            hsl = slice(half * 512, (half + 1) * 512)
                            xsl = slice(sbp * QH + half * 512, sbp * QH + (half + 1) * 512)
                            nc.tensor.matmul(
                                psA[:, hsl], wb[:, d, 0:P], xt_d[:, xsl],
                                start=(d == 0), stop=(d == ND - 1),
                            )
                    for d in range(ND):
                        xt_d = xtc[(name, d)]
                        for half in range(2):
                            hsl = slice(half * 512, (half + 1) * 512)
                            xsl = slice(sbp * QH + half * 512, sbp * QH + (half + 1) * 512)
                            nc.tensor.matmul(
                                psB[:, hsl], wb[:, d, P:HD], xt_d[:, xsl],
                                start=(d == 0), stop=(d == ND - 1),
                            )
                    if use_bias_qkv:
                        ba, bb = bias_a[name], bias_b[name]
                        nc.vector.tensor_scalar_add(dst[0:DK, 0, sl], psA[0:DK, :], ba[0:DK])
                        nc.vector.tensor_scalar_add(dst[0:DK, 1, sl], psA[DK:P, :], ba[DK:P])
                        nc.vector.tensor_scalar_add(dst[0:DK, 2, sl], psB, bb)
                    else:
                        nc.vector.tensor_copy(out=dst[0:DK, 0, sl], in_=psA[0:DK, :])
                        nc.vector.tensor_copy(out=dst[0:DK, 1, sl], in_=psA[DK:P, :])
                        nc.vector.tensor_copy(out=dst[0:DK, 2, sl], in_=psB)

            qk_proj("wk", KT)
            qk_proj("wq", QT)

            wb = w_bf["wv"]
            for st in range(NS):
                psV = psv_pool.tile([P, HD], F32, tag="psv", name=f"psV{st}")
                for d in range(ND):
                    nc.tensor.matmul(
                        psV, xtc[("wv", d)][:, st * P : (st + 1) * P], wb[:, d, :],
                        start=(d == 0), stop=(d == ND - 1),
                    )
                for h in range(HG):
                    if use_bias_qkv:
                        nc.vector.tensor_add(
                            V[:, st, h * 65 : h * 65 + 64],
                            psV[:, h * DK : (h + 1) * DK],
                            bias_vrow[:, h * DK : (h + 1) * DK],
                        )
                    else:
                        nc.vector.tensor_copy(
                            out=V[:, st, h * 65 : h * 65 + 64],
                            in_=psV[:, h * DK : (h + 1) * DK],
                        )

        # ============ phase 2: attention (+ wo) ============
        with tc.tile_pool(name="s_ps", bufs=2, space="PSUM") as s_pool, \
             tc.tile_pool(name="ot_ps", bufs=2, space="PSUM") as ot_pool, \
             tc.tile_pool(name="pt", bufs=4) as pt_pool, \
             tc.tile_pool(name="nrm", bufs=2) as nrm_pool, \
             tc.tile_pool(name="y_sb", bufs=2) as ysb_pool:

            # qh-major so wo for q-half 0 can run before the final phase
            phases = [(h, 0) for h in range(HG)] + [(h, 1) for h in range(HG)]
            ots = {}
            pts = {}

            def s_exp(i):
                h, qh = phases[i // NS]
                kt = i % NS
                s_ps = s_pool.tile([P, QH], F32, tag="s", name=f"s{i}")
                for n in range(2):
                    q0 = qh * QH + n * 512
                    nc.tensor.matmul(
                        s_ps[:, n * 512 : (n + 1) * 512],
                        KT[:, h, kt * P : (kt + 1) * P],
                        QT[:, h, q0 : q0 + 512],
                        start=True, stop=True,
                    )
                pt = pt_pool.tile([P, QH], BF, tag="pt", name=f"pt{i}")
                nc.scalar.activation(pt, s_ps, EXP, bias=0.0, scale=0.125)
                pts[i] = pt

            def pv(i):
                h, qh = phases[i // NS]
                kt = i % NS
                if kt == 0:
                    ots[(h, qh)] = ot_pool.tile([65, QH], F32, tag="ot", name=f"ot{h}_{qh}")
                ot = ots[(h, qh)]
                pt = pts.pop(i)
                for n in range(2):
                    nc.tensor.matmul(
                        ot[:, n * 512 : (n + 1) * 512],
                        V[:, kt, h * 65 : (h + 1) * 65],
                        pt[:, n * 512 : (n + 1) * 512],
                        start=(kt == 0), stop=(kt == NS - 1),
                    )

            def normalize(h, qh):
                ot = ots.pop((h, qh))
                osb = nrm_pool.tile([DK, QH], F32, tag="osb", name="osb")
                nc.vector.tensor_copy(out=osb, in_=ot[0:DK, :])
                den = nrm_pool.tile([1, QH], F32, tag="den", name="den")
                nc.scalar.copy(den, ot[64:65, :])
                recip = nrm_pool.tile([1, QH], F32, tag="recip", name="recip")
                nc.vector.reciprocal_approx_fast(recip, den)
                rbc = nrm_pool.tile([DK, QH], F32, tag="rbc", name="rbc")
                nc.gpsimd.partition_broadcast(rbc, recip)
                sl = slice(qh * QH, (qh + 1) * QH)
                dst = OC1[0:DK, sl] if h == 0 else (OC1[DK:P, sl] if h == 1 else OC2[0:DK, sl])
                nc.vector.tensor_mul(dst, osb, rbc)

            y_r = y.rearrange("(n p) m -> n p m", p=P)

            def writeback(dst, y_ps, st, tail=False):
                y_sb = ysb_pool.tile([P, D], BF, tag="ysb", name=f"y_sb{st}")
                nc.vector.tensor_copy(out=y_sb, in_=y_ps)
                nc.sync.dma_start(out=dst, in_=y_sb)

            def wo_parts(y_ps, st, parts):
                sl = slice(st * P, (st + 1) * P)
                # region-major order: adjacent matmuls never accumulate into
                # the same PSUM region (back-to-back same-bank accumulation
                # serializes the PE)
                last = len(parts) - 1
                for pi, (oc, wob) in enumerate(parts):
                    for n0, nn in ((0, 512), (512, 256)):
                        nc.tensor.matmul(
                            y_ps[:, n0 : n0 + nn], oc[:, sl], wob[:, n0 : n0 + nn],
                            start=(pi == 0), stop=(pi == last),
                        )

            def wo_tile(st, tail=False):
                y_ps = ot_pool.tile([P, D], F32, tag="ot", name=f"y_ps{st}")
                wo_parts(y_ps, st, [(OC1, wo_b1), (OC2, wo_b2)])
                writeback(y_r[st], y_ps, st, tail=tail)

            n_steps = len(phases) * NS
            s_exp(0)
            s_exp(1)
            wo_pending = []
            wo_emitted = 0
            for i in range(n_steps):
                if i + 2 < n_steps:
                    s_exp(i + 2)
                pv(i)
                if wo_pending:
                    # spread q-half-0 wo tiles one per step instead of a
                    # blocking burst
                    wo_tile(wo_pending.pop(0))
                    wo_emitted += 1
                h, qh = phases[i // NS]
                kt = i % NS
                if kt == NS - 1:
                    normalize(h, qh)
                    if (h, qh) == (HG - 1, 0):
                        wo_pending = list(range(NS // 2))
            for st in range(wo_emitted, NS):
                wo_tile(st, tail=True)

    nc.compile()
    return nc


def kernel(query, key, value, Wq, bq, Wk, bk, Wv, bv, Wo, bo, **_ignored):
    import ml_dtypes
    from concourse.bass_utils import run_bass_kernel_spmd

    bf16 = ml_dtypes.bfloat16
    query = np.asarray(query, dtype=np.float32)
    key = np.asarray(key, dtype=np.float32)
    value = np.asarray(value, dtype=np.float32)
    Wq = np.asarray(Wq, dtype=np.float32)
    Wk = np.asarray(Wk, dtype=np.float32)
    Wv = np.asarray(Wv, dtype=np.float32)
    Wo = np.asarray(Wo, dtype=np.float32)
    bq = np.asarray(bq, dtype=np.float32)
    bk = np.asarray(bk, dtype=np.float32)
    bv = np.asarray(bv, dtype=np.float32)
    bo = np.asarray(bo, dtype=np.float32)

    use_bias_qkv = bool(np.any(bq) or np.any(bk) or np.any(bv))
    if "nc" not in _CACHE or _CACHE.get("bias") != use_bias_qkv:
        _CACHE["nc"] = _build_nc(use_bias_qkv)
        _CACHE["bias"] = use_bias_qkv
    nc = _CACHE["nc"]

    xT = {b: {} for b in range(B)}
    for b in range(B):
        xT[b]["q"] = np.ascontiguousarray(query[b].T).astype(bf16)
        xT[b]["k"] = np.ascontiguousarray(key[b].T).astype(bf16)
        xT[b]["v"] = np.ascontiguousarray(value[b].T).astype(bf16)

    in_maps = []
    for c in range(8):
        b, g = divmod(c, 4)
        hs = slice(g * HD, (g + 1) * HD)
        in_maps.append({
            "xqT": xT[b]["q"],
            "xkT": xT[b]["k"],
            "xvT": xT[b]["v"],
            "wq": np.ascontiguousarray(Wq[:, hs]).astype(bf16),
            "wk": np.ascontiguousarray(Wk[:, hs]).astype(bf16),
            "wv": np.ascontiguousarray(Wv[:, hs]).astype(bf16),
            "wo": np.ascontiguousarray(Wo[hs, :]).astype(bf16),
            "bqkv": np.ascontiguousarray(
                np.stack([bq[hs], bk[hs], bv[hs]]).astype(np.float32)
            ),
        })

    res = run_bass_kernel_spmd(nc, in_maps, core_ids=list(range(8)), **_CACHE.get("run_kwargs", {}))
    _CACHE["last_result"] = res

    out = np.empty((B, S, D), dtype=np.float32)
    for b in range(B):
        acc = res.results[4 * b]["y"].astype(np.float32)
        for g in range(1, 4):
            acc = acc + res.results[4 * b + g]["y"].astype(np.float32)
        out[b] = acc + bo[None, :]
    return out


# revision 30
# speedup vs baseline: 1.1219x; 1.0081x over previous
"""Multi-head attention (B=2, S=2048, D=768, H=12) on 8 Trainium2 cores.

Sharding: core c -> batch b = c // 4, head-group g = c % 4 (3 heads of 12).
Host prep: x^T per batch pre-transposed AND cast to bf16 (halves the x DMA
vs fp32+casting-DMA); weight shards cast to bf16.  Each core projects
Q/K/V for its 3 heads, runs attention, emits its Wo row-shard partial as
bf16; the host sums 4 partials per batch in fp32 and adds bo.

Device kernel (per core):
  - Q^T/K^T stored zero-PADDED to 128 contraction rows ([128, 3, S] tiles,
    rows 64-127 = 0) so every scores matmul is a full 128x128-array
    instruction: the 64-row (half-array / HAM k=4) config measured ~2x
    slower sustained on HW (activity throttle), and padding costs no extra
    PE cycles (row count = rhs free size).
  - All matmul outputs are <=512 fp32 columns (one PSUM bank; 1024-col out
    is an ISA violation, probed).
  - Attention runs as 6 phases (qh-major: (h0,h1,h2) x qh0 then qh1), each
    16 kt steps of: scores 2mm -> exp (ScalarE, [128,1024] tiles) -> PV
    2mm accumulating [65,1024] (ones column in V_aug rides the softmax
    denominator).  A global 2-step software pipeline (scores of step i+2
    emitted before PV of step i) keeps the in-order PE queue from ever
    waiting on the ACT exp, across phase boundaries too.
  - Wo tiles for the first q-half are emitted right after (h2,qh0)'s
    normalize, shortening the serial tail to normalize + 8 wo tiles.
"""

import sys

for _p in ("/opt/trn_rl_repo",):
    if _p not in sys.path:
        sys.path.append(_p)

import numpy as np

B = 2
S = 2048
D = 768
H = 12
DK = 64
HG = 3            # heads per core
HD = HG * DK      # 192
P = 128
NS = S // P       # 16 k-tiles
ND = D // P       # 6 d-chunks
QH = 1024         # q half

_CACHE = {}


def _build_nc(use_bias_qkv):
    import concourse.bacc as bacc
    import concourse.tile as tile
    from concourse import mybir
    from contextlib import ExitStack

    BF = mybir.dt.bfloat16
    F32 = mybir.dt.float32
    EXP = mybir.ActivationFunctionType.Exp

    nc = bacc.Bacc("TRN2", target_bir_lowering=False, debug=False)

    xqT = nc.dram_tensor("xqT", [D, S], BF, kind="ExternalInput").ap()
    xkT = nc.dram_tensor("xkT", [D, S], BF, kind="ExternalInput").ap()
    xvT = nc.dram_tensor("xvT", [D, S], BF, kind="ExternalInput").ap()
    wq = nc.dram_tensor("wq", [D, HD], BF, kind="ExternalInput").ap()
    wk = nc.dram_tensor("wk", [D, HD], BF, kind="ExternalInput").ap()
    wv = nc.dram_tensor("wv", [D, HD], BF, kind="ExternalInput").ap()
    wo = nc.dram_tensor("wo", [HD, D], BF, kind="ExternalInput").ap()
    bqkv = nc.dram_tensor("bqkv", [3, HD], F32, kind="ExternalInput").ap()
    y = nc.dram_tensor("y", [S, D], BF, kind="ExternalOutput").ap()

    with tile.TileContext(nc) as tc, ExitStack() as ctx:
        wpool = ctx.enter_context(tc.tile_pool(name="weights", bufs=1))
        apool = ctx.enter_context(tc.tile_pool(name="acts", bufs=1))

        # zero-padded transposed activations: [:, h, :] = head h, rows 64+ = 0
        KT = apool.tile([P, HG, S], BF, tag="kt")
        QT = apool.tile([P, HG, S], BF, tag="qt")
        V = apool.tile([P, NS, 3 * 65], BF, tag="v")
        OC1 = apool.tile([P, S], BF, tag="oc1")    # heads 0,1 of O^T (normalized)
        OC2 = apool.tile([P, S], BF, tag="oc2")    # head 2, rows 64-127 = 0 (keeps
                                                   # the wo matmuls in full-array config)

        # x chunk tiles (bf16 straight from HBM), all resident
        # x chunks DMA'd in s-halves, first halves of all d-chunks first, so
        # the sbp0 projections can start ~4us after the tensor's DMA begins
        xt_pool = ctx.enter_context(tc.tile_pool(name="xt", bufs=1))
        xtc = {}
        for name, xT in (("wk", xkT), ("wq", xqT), ("wv", xvT)):
            for dc in range(ND):
                xtc[(name, dc)] = xt_pool.tile(
                    [P, S], BF, tag=f"xt_{name}{dc}", name=f"xt_{name}{dc}"
                )
            for half in range(2):
                hsl = slice(half * QH, (half + 1) * QH)
                for dc in range(ND):
                    nc.gpsimd.dma_start(
                        out=xtc[(name, dc)][:, hsl],
                        in_=xT[dc * P : (dc + 1) * P, hsl],
                    )

        # weights (bf16 on host, no device casts), HWDGE queue
        w_bf = {}
        for name, w in (("wk", wk), ("wq", wq), ("wv", wv)):
            wb = wpool.tile([P, ND, HD], BF, tag=f"{name}_bf", name=f"{name}_bf")
            nc.sync.dma_start(out=wb, in_=w.rearrange("(nd p) h -> p nd h", p=P))
            w_bf[name] = wb
        wo_b1 = wpool.tile([P, D], BF, tag="wo_b1")
        nc.sync.dma_start(out=wo_b1, in_=wo[0:P, :])
        wo_b2 = wpool.tile([P, D], BF, tag="wo_b2")   # rows 64-127 = 0 (padding)
        nc.sync.dma_start(out=wo_b2[0:DK, :], in_=wo[P:HD, :])

        bias_a = {}
        bias_b = {}
        bias_vrow = None
        if use_bias_qkv:
            for i, name in enumerate(("wq", "wk", "wv")):
                ba = wpool.tile([P, 1], F32, tag=f"ba_{name}", name=f"ba_{name}")
                nc.sync.dma_start(out=ba, in_=bqkv[i, 0:P].rearrange("p -> p 1"))
                bb = wpool.tile([DK, 1], F32, tag=f"bb_{name}", name=f"bb_{name}")
                nc.sync.dma_start(out=bb, in_=bqkv[i, P:HD].rearrange("p -> p 1"))
                bias_a[name] = ba
                bias_b[name] = bb
            # V bias varies along the free dim of psV [s, 192]: broadcast the
            # bias row across all 128 partitions once
            bvr = wpool.tile([1, HD], F32, tag="bv_row")
            nc.sync.dma_start(out=bvr, in_=bqkv[2, :].rearrange("h -> 1 h"))
            bias_vrow = wpool.tile([P, HD], F32, tag="bv_bcast")
            nc.gpsimd.partition_broadcast(bias_vrow, bvr)

        # padding zeros + V ones columns (off the PE path; after DMA triggers)
        nc.gpsimd.memset(KT[DK:P, :, :], 0.0)
        nc.vector.memset(QT[DK:P, :, :], 0.0)
        nc.vector.memset(V[:, :, 64 : 3 * 65 : 65], 1.0)
        nc.gpsimd.memset(OC2[DK:P, :], 0.0)
        nc.vector.memset(wo_b2[DK:P, :], 0.0)

        # ================= phase 1: projections =================
        with tc.tile_pool(name="ppa", bufs=2, space="PSUM") as ppa_pool, \
             tc.tile_pool(name="ppb", bufs=1, space="PSUM") as ppb_pool, \
             tc.tile_pool(name="psv", bufs=2, space="PSUM") as psv_pool:

            def qk_proj(name, dst):
                wb = w_bf[name]
                for sbp in range(2):
                    sl = slice(sbp * QH, (sbp + 1) * QH)
                    psA = ppa_pool.tile([P, QH], F32, tag="ppa", name=f"psA_{name}{sbp}")
                    psB = ppb_pool.tile([DK, QH], F32, tag="ppb", name=f"psB_{name}{sbp}")
                    for d in range(ND):
                        xt_d = xtc[(name, d)]
                        for half in range(2):
                            hsl = slice(half * 512, (half + 1) * 512)
                            xsl = slice(sbp * QH + half * 512, sbp * QH + (half + 1) * 512)
                            nc.tensor.matmul(
                                psA[:, hsl], wb[:, d, 0:P], xt_d[:, xsl],
                                start=(d == 0), stop=(d == ND - 1),
                            )
                    for d in range(ND):
                        xt_d = xtc[(name, d)]
                        for half in range(2):
                            hsl = slice(half * 512, (half + 1) * 512)
                            xsl = slice(sbp * QH + half * 512, sbp * QH + (half + 1) * 512)
                            nc.tensor.matmul(
                                psB[:, hsl], wb[:, d, P:HD], xt_d[:, xsl],
                                start=(d == 0), stop=(d == ND - 1),
                            )
                    if use_bias_qkv:
                        ba, bb = bias_a[name], bias_b[name]
                        nc.vector.tensor_scalar_add(dst[0:DK, 0, sl], psA[0:DK, :], ba[0:DK])
                        nc.vector.tensor_scalar_add(dst[0:DK, 1, sl], psA[DK:P, :], ba[DK:P])
                        nc.vector.tensor_scalar_add(dst[0:DK, 2, sl], psB, bb)
                    else:
                        nc.vector.tensor_copy(out=dst[0:DK, 0, sl], in_=psA[0:DK, :])
                        nc.vector.tensor_copy(out=dst[0:DK, 1, sl], in_=psA[DK:P, :])
                        nc.vector.tensor_copy(out=dst[0:DK, 2, sl], in_=psB)

            qk_proj("wk", KT)
            qk_proj("wq", QT)

            wb = w_bf["wv"]
            for st in range(NS):
                psV = psv_pool.tile([P, HD], F32, tag="psv", name=f"psV{st}")
                for d in range(ND):
                    nc.tensor.matmul(
                        psV, xtc[("wv", d)][:, st * P : (st + 1) * P], wb[:, d, :],
                        start=(d == 0), stop=(d == ND - 1),
                    )
                for h in range(HG):
                    if use_bias_qkv:
                        nc.vector.tensor_add(
                            V[:, st, h * 65 : h * 65 + 64],
                            psV[:, h * DK : (h + 1) * DK],
                            bias_vrow[:, h * DK : (h + 1) * DK],
                        )
                    else:
                        nc.vector.tensor_copy(
                            out=V[:, st, h * 65 : h * 65 + 64],
                            in_=psV[:, h * DK : (h + 1) * DK],
                        )

        # ============ phase 2: attention (+ wo) ============
        with tc.tile_pool(name="s_ps", bufs=2, space="PSUM") as s_pool, \
             tc.tile_pool(name="ot_ps", bufs=2, space="PSUM") as ot_pool, \
             tc.tile_pool(name="pt", bufs=4) as pt_pool, \
             tc.tile_pool(name="nrm", bufs=2) as nrm_pool, \
             tc.tile_pool(name="y_sb", bufs=2) as ysb_pool:

            # qh-major so wo for q-half 0 can run before the final phase
            phases = [(h, 0) for h in range(HG)] + [(h, 1) for h in range(HG)]
            ots = {}
            pts = {}

            def s_exp(i):
                h, qh = phases[i // NS]
                kt = i % NS
                s_ps = s_pool.tile([P, QH], F32, tag="s", name=f"s{i}")
                for n in range(2):
                    q0 = qh * QH + n * 512
                    nc.tensor.matmul(
                        s_ps[:, n * 512 : (n + 1) * 512],
                        KT[:, h, kt * P : (kt + 1) * P],
                        QT[:, h, q0 : q0 + 512],
                        start=True, stop=True,
                    )
                pt = pt_pool.tile([P, QH], BF, tag="pt", name=f"pt{i}")
                nc.scalar.activation(pt, s_ps, EXP, bias=0.0, scale=0.125)
                pts[i] = pt

            def pv(i):
                h, qh = phases[i // NS]
                kt = i % NS
                if kt == 0:
                    ots[(h, qh)] = ot_pool.tile([65, QH], F32, tag="ot", name=f"ot{h}_{qh}")
                ot = ots[(h, qh)]
                pt = pts.pop(i)
                for n in range(2):
                    nc.tensor.matmul(
                        ot[:, n * 512 : (n + 1) * 512],
                        V[:, kt, h * 65 : (h + 1) * 65],
                        pt[:, n * 512 : (n + 1) * 512],
                        start=(kt == 0), stop=(kt == NS - 1),
                    )

            def normalize(h, qh):
                ot = ots.pop((h, qh))
                osb = nrm_pool.tile([DK, QH], F32, tag="osb", name="osb")
                nc.vector.tensor_copy(out=osb, in_=ot[0:DK, :])
                den = nrm_pool.tile([1, QH], F32, tag="den", name="den")
                nc.scalar.copy(den, ot[64:65, :])
                recip = nrm_pool.tile([1, QH], F32, tag="recip", name="recip")
                nc.vector.reciprocal_approx_fast(recip, den)
                rbc = nrm_pool.tile([DK, QH], F32, tag="rbc", name="rbc")
                nc.gpsimd.partition_broadcast(rbc, recip)
                sl = slice(qh * QH, (qh + 1) * QH)
                dst = OC1[0:DK, sl] if h == 0 else (OC1[DK:P, sl] if h == 1 else OC2[0:DK, sl])
                nc.vector.tensor_mul(dst, osb, rbc)

            y_r = y.rearrange("(n p) m -> n p m", p=P)

            def writeback(dst, y_ps, st, tail=False):
                y_sb = ysb_pool.tile([P, D], BF, tag="ysb", name=f"y_sb{st}")
                # in the tail the exp stream is done, so ScalarE is free:
                # alternate the PSUM->SBUF cast across vector/scalar so the
                # eight final writebacks drain at 2x
                if tail and st % 2 == 1:
                    nc.scalar.copy(y_sb, y_ps)
                else:
                    nc.vector.tensor_copy(out=y_sb, in_=y_ps)
                nc.sync.dma_start(out=dst, in_=y_sb)

            def wo_parts(y_ps, st, parts):
                sl = slice(st * P, (st + 1) * P)
                # region-major order: adjacent matmuls never accumulate into
                # the same PSUM region (back-to-back same-bank accumulation
                # serializes the PE)
                last = len(parts) - 1
                for pi, (oc, wob) in enumerate(parts):
                    for n0, nn in ((0, 512), (512, 256)):
                        nc.tensor.matmul(
                            y_ps[:, n0 : n0 + nn], oc[:, sl], wob[:, n0 : n0 + nn],
                            start=(pi == 0), stop=(pi == last),
                        )

            def wo_tile(st, tail=False):
                y_ps = ot_pool.tile([P, D], F32, tag="ot", name=f"y_ps{st}")
                wo_parts(y_ps, st, [(OC1, wo_b1), (OC2, wo_b2)])
                writeback(y_r[st], y_ps, st, tail=tail)

            n_steps = len(phases) * NS
            s_exp(0)
            s_exp(1)
            wo_pending = []
            wo_emitted = 0
            for i in range(n_steps):
                if i + 2 < n_steps:
                    s_exp(i + 2)
                pv(i)
                if wo_pending:
                    # spread q-half-0 wo tiles one per step instead of a
                    # blocking burst
                    wo_tile(wo_pending.pop(0))
                    wo_emitted += 1
                h, qh = phases[i // NS]
                kt = i % NS
                if kt == NS - 1:
                    normalize(h, qh)
                    if (h, qh) == (HG - 1, 0):
                        wo_pending = list(range(NS // 2))
            for st in range(wo_emitted, NS):
                wo_tile(st, tail=True)

    nc.compile()
    return nc


def kernel(query, key, value, Wq, bq, Wk, bk, Wv, bv, Wo, bo, **_ignored):
    import ml_dtypes
    from concourse.bass_utils import run_bass_kernel_spmd

    bf16 = ml_dtypes.bfloat16
    query = np.asarray(query, dtype=np.float32)
    key = np.asarray(key, dtype=np.float32)
    value = np.asarray(value, dtype=np.float32)
    Wq = np.asarray(Wq, dtype=np.float32)
    Wk = np.asarray(Wk, dtype=np.float32)
    Wv = np.asarray(Wv, dtype=np.float32)
    Wo = np.asarray(Wo, dtype=np.float32)
    bq = np.asarray(bq, dtype=np.float32)
    bk = np.asarray(bk, dtype=np.float32)
    bv = np.asarray(bv, dtype=np.float32)
    bo = np.asarray(bo, dtype=np.float32)

    use_bias_qkv = bool(np.any(bq) or np.any(bk) or np.any(bv))
    if "nc" not in _CACHE or _CACHE.get("bias") != use_bias_qkv:
        _CACHE["nc"] = _build_nc(use_bias_qkv)
        _CACHE["bias"] = use_bias_qkv
    nc = _CACHE["nc"]

    xT = {b: {} for b in range(B)}
    for b in range(B):
        xT[b]["q"] = np.ascontiguousarray(query[b].T).astype(bf16)
        xT[b]["k"] = np.ascontiguousarray(key[b].T).astype(bf16)
        xT[b]["v"] = np.ascontiguousarray(value[b].T).astype(bf16)

    in_maps = []
    for c in range(8):
        b, g = divmod(c, 4)
        hs = slice(g * HD, (g + 1) * HD)
        in_maps.append({
            "xqT": xT[b]["q"],
            "xkT": xT[b]["k"],
            "xvT": xT[b]["v"],
            "wq": np.ascontiguousarray(Wq[:, hs]).astype(bf16),
            "wk": np.ascontiguousarray(Wk[:, hs]).astype(bf16),
            "wv": np.ascontiguousarray(Wv[:, hs]).astype(bf16),
            "wo": np.ascontiguousarray(Wo[hs, :]).astype(bf16),
            "bqkv": np.ascontiguousarray(
                np.stack([bq[hs], bk[hs], bv[hs]]).astype(np.float32)
            ),
        })

    res = run_bass_kernel_spmd(nc, in_maps, core_ids=list(range(8)), **_CACHE.get("run_kwargs", {}))
    _CACHE["last_result"] = res

    out = np.empty((B, S, D), dtype=np.float32)
    for b in range(B):
        acc = res.results[4 * b]["y"].astype(np.float32)
        for g in range(1, 4):
            acc = acc + res.results[4 * b + g]["y"].astype(np.float32)
        out[b] = acc + bo[None, :]
    return out


# revision 32
# speedup vs baseline: 1.1378x; 1.0142x over previous
"""Multi-head attention (B=2, S=2048, D=768, H=12) on 8 Trainium2 cores.

Sharding: core c -> batch b = c // 4, head-group g = c % 4 (3 heads of 12).
Host prep: x^T per batch pre-transposed AND cast to bf16 (halves the x DMA
vs fp32+casting-DMA); weight shards cast to bf16.  Each core projects
Q/K/V for its 3 heads, runs attention, emits its Wo row-shard partial as
bf16; the host sums 4 partials per batch in fp32 and adds bo.

Device kernel (per core):
  - Q^T/K^T stored zero-PADDED to 128 contraction rows ([128, 3, S] tiles,
    rows 64-127 = 0) so every scores matmul is a full 128x128-array
    instruction: the 64-row (half-array / HAM k=4) config measured ~2x
    slower sustained on HW (activity throttle), and padding costs no extra
    PE cycles (row count = rhs free size).
  - All matmul outputs are <=512 fp32 columns (one PSUM bank; 1024-col out
    is an ISA violation, probed).
  - Attention runs as 6 phases (qh-major: (h0,h1,h2) x qh0 then qh1), each
    16 kt steps of: scores 2mm -> exp (ScalarE, [128,1024] tiles) -> PV
    2mm accumulating [65,1024] (ones column in V_aug rides the softmax
    denominator).  A global 2-step software pipeline (scores of step i+2
    emitted before PV of step i) keeps the in-order PE queue from ever
    waiting on the ACT exp, across phase boundaries too.
  - Wo tiles for the first q-half are emitted right after (h2,qh0)'s
    normalize, shortening the serial tail to normalize + 8 wo tiles.
"""

import sys

for _p in ("/opt/trn_rl_repo",):
    if _p not in sys.path:
        sys.path.append(_p)

import numpy as np

B = 2
S = 2048
D = 768
H = 12
DK = 64
HG = 3            # heads per core
HD = HG * DK      # 192
P = 128
NS = S // P       # 16 k-tiles
ND = D // P       # 6 d-chunks
QH = 1024         # q half

_CACHE = {}


def _build_nc(use_bias_qkv):
    import concourse.bacc as bacc
    import concourse.tile as tile
    from concourse import mybir
    from contextlib import ExitStack

    BF = mybir.dt.bfloat16
    F32 = mybir.dt.float32
    EXP = mybir.ActivationFunctionType.Exp

    nc = bacc.Bacc("TRN2", target_bir_lowering=False, debug=False)

    xqT = nc.dram_tensor("xqT", [D, S], BF, kind="ExternalInput").ap()
    xkT = nc.dram_tensor("xkT", [D, S], BF, kind="ExternalInput").ap()
    xvT = nc.dram_tensor("xvT", [D, S], BF, kind="ExternalInput").ap()
    wq = nc.dram_tensor("wq", [D, HD], BF, kind="ExternalInput").ap()
    wk = nc.dram_tensor("wk", [D, HD], BF, kind="ExternalInput").ap()
    wv = nc.dram_tensor("wv", [D, HD], BF, kind="ExternalInput").ap()
    wo = nc.dram_tensor("wo", [HD, D], BF, kind="ExternalInput").ap()
    bqkv = nc.dram_tensor("bqkv", [3, HD], F32, kind="ExternalInput").ap()
    y = nc.dram_tensor("y", [S, D], BF, kind="ExternalOutput").ap()

    with tile.TileContext(nc) as tc, ExitStack() as ctx:
        wpool = ctx.enter_context(tc.tile_pool(name="weights", bufs=1))
        apool = ctx.enter_context(tc.tile_pool(name="acts", bufs=1))

        # zero-padded transposed activations: [:, h, :] = head h, rows 64+ = 0
        KT = apool.tile([P, HG, S], BF, tag="kt")
        QT = apool.tile([P, HG, S], BF, tag="qt")
        V = apool.tile([P, NS, 3 * 65], BF, tag="v")
        OC1 = apool.tile([P, S], BF, tag="oc1")    # heads 0,1 of O^T (normalized)
        OC2 = apool.tile([P, S], BF, tag="oc2")    # head 2, rows 64-127 = 0 (keeps
                                                   # the wo matmuls in full-array config)

        # x chunk tiles (bf16 straight from HBM), all resident
        # x chunks DMA'd in s-halves, first halves of all d-chunks first, so
        # the sbp0 projections can start ~4us after the tensor's DMA begins
        xt_pool = ctx.enter_context(tc.tile_pool(name="xt", bufs=1))
        xtc = {}
        for name, xT in (("wk", xkT), ("wq", xqT), ("wv", xvT)):
            for dc in range(ND):
                xtc[(name, dc)] = xt_pool.tile(
                    [P, S], BF, tag=f"xt_{name}{dc}", name=f"xt_{name}{dc}"
                )
            for half in range(2):
                hsl = slice(half * QH, (half + 1) * QH)
                for dc in range(ND):
                    nc.gpsimd.dma_start(
                        out=xtc[(name, dc)][:, hsl],
                        in_=xT[dc * P : (dc + 1) * P, hsl],
                    )

        # weights (bf16 on host, no device casts), HWDGE queue
        w_bf = {}
        for name, w in (("wk", wk), ("wq", wq), ("wv", wv)):
            wb = wpool.tile([P, ND, HD], BF, tag=f"{name}_bf", name=f"{name}_bf")
            nc.sync.dma_start(out=wb, in_=w.rearrange("(nd p) h -> p nd h", p=P))
            w_bf[name] = wb
        wo_b1 = wpool.tile([P, D], BF, tag="wo_b1")
        nc.sync.dma_start(out=wo_b1, in_=wo[0:P, :])
        wo_b2 = wpool.tile([P, D], BF, tag="wo_b2")   # rows 64-127 = 0 (padding)
        nc.sync.dma_start(out=wo_b2[0:DK, :], in_=wo[P:HD, :])

        bias_a = {}
        bias_b = {}
        bias_vrow = None
        if use_bias_qkv:
            for i, name in enumerate(("wq", "wk", "wv")):
                ba = wpool.tile([P, 1], F32, tag=f"ba_{name}", name=f"ba_{name}")
                nc.sync.dma_start(out=ba, in_=bqkv[i, 0:P].rearrange("p -> p 1"))
                bb = wpool.tile([DK, 1], F32, tag=f"bb_{name}", name=f"bb_{name}")
                nc.sync.dma_start(out=bb, in_=bqkv[i, P:HD].rearrange("p -> p 1"))
                bias_a[name] = ba
                bias_b[name] = bb
            # V bias varies along the free dim of psV [s, 192]: broadcast the
            # bias row across all 128 partitions once
            bvr = wpool.tile([1, HD], F32, tag="bv_row")
            nc.sync.dma_start(out=bvr, in_=bqkv[2, :].rearrange("h -> 1 h"))
            bias_vrow = wpool.tile([P, HD], F32, tag="bv_bcast")
            nc.gpsimd.partition_broadcast(bias_vrow, bvr)

        # padding zeros + V ones columns (off the PE path; after DMA triggers)
        nc.gpsimd.memset(KT[DK:P, :, :], 0.0)
        nc.vector.memset(QT[DK:P, :, :], 0.0)
        nc.vector.memset(V[:, :, 64 : 3 * 65 : 65], 1.0)
        nc.gpsimd.memset(OC2[DK:P, :], 0.0)
        nc.vector.memset(wo_b2[DK:P, :], 0.0)

        # ================= phase 1: projections =================
        with tc.tile_pool(name="ppa", bufs=2, space="PSUM") as ppa_pool, \
             tc.tile_pool(name="ppb", bufs=1, space="PSUM") as ppb_pool, \
             tc.tile_pool(name="psv", bufs=2, space="PSUM") as psv_pool:

            def qk_proj(name, dst):
                wb = w_bf[name]
                for sbp in range(2):
                    sl = slice(sbp * QH, (sbp + 1) * QH)
                    psA = ppa_pool.tile([P, QH], F32, tag="ppa", name=f"psA_{name}{sbp}")
                    psB = ppb_pool.tile([DK, QH], F32, tag="ppb", name=f"psB_{name}{sbp}")
                    for d in range(ND):
                        xt_d = xtc[(name, d)]
                        for half in range(2):
                            hsl = slice(half * 512, (half + 1) * 512)
                            xsl = slice(sbp * QH + half * 512, sbp * QH + (half + 1) * 512)
                            nc.tensor.matmul(
                                psA[:, hsl], wb[:, d, 0:P], xt_d[:, xsl],
                                start=(d == 0), stop=(d == ND - 1),
                            )
                    for d in range(ND):
                        xt_d = xtc[(name, d)]
                        for half in range(2):
                            hsl = slice(half * 512, (half + 1) * 512)
                            xsl = slice(sbp * QH + half * 512, sbp * QH + (half + 1) * 512)
                            nc.tensor.matmul(
                                psB[:, hsl], wb[:, d, P:HD], xt_d[:, xsl],
                                start=(d == 0), stop=(d == ND - 1),
                            )
                    if use_bias_qkv:
                        ba, bb = bias_a[name], bias_b[name]
                        nc.vector.tensor_scalar_add(dst[0:DK, 0, sl], psA[0:DK, :], ba[0:DK])
                        nc.vector.tensor_scalar_add(dst[0:DK, 1, sl], psA[DK:P, :], ba[DK:P])
                        nc.vector.tensor_scalar_add(dst[0:DK, 2, sl], psB, bb)
                    else:
                        nc.vector.tensor_copy(out=dst[0:DK, 0, sl], in_=psA[0:DK, :])
                        nc.vector.tensor_copy(out=dst[0:DK, 1, sl], in_=psA[DK:P, :])
                        nc.vector.tensor_copy(out=dst[0:DK, 2, sl], in_=psB)

            qk_proj("wk", KT)
            qk_proj("wq", QT)

            wb = w_bf["wv"]
            for st in range(NS):
                psV = psv_pool.tile([P, HD], F32, tag="psv", name=f"psV{st}")
                for d in range(ND):
                    nc.tensor.matmul(
                        psV, xtc[("wv", d)][:, st * P : (st + 1) * P], wb[:, d, :],
                        start=(d == 0), stop=(d == ND - 1),
                    )
                for h in range(HG):
                    if use_bias_qkv:
                        nc.vector.tensor_add(
                            V[:, st, h * 65 : h * 65 + 64],
                            psV[:, h * DK : (h + 1) * DK],
                            bias_vrow[:, h * DK : (h + 1) * DK],
                        )
                    else:
                        nc.vector.tensor_copy(
                            out=V[:, st, h * 65 : h * 65 + 64],
                            in_=psV[:, h * DK : (h + 1) * DK],
                        )

        # ============ phase 2: attention (+ wo) ============
        with tc.tile_pool(name="s_ps", bufs=2, space="PSUM") as s_pool, \
             tc.tile_pool(name="ot_ps", bufs=2, space="PSUM") as ot_pool, \
             tc.tile_pool(name="pt", bufs=4) as pt_pool, \
             tc.tile_pool(name="nrm", bufs=2) as nrm_pool, \
             tc.tile_pool(name="y_sb", bufs=2) as ysb_pool:

            # qh-major so wo for q-half 0 can run before the final phase
            phases = [(h, 0) for h in range(HG)] + [(h, 1) for h in range(HG)]
            ots = {}
            pts = {}

            def s_exp(i):
                h, qh = phases[i // NS]
                kt = i % NS
                s_ps = s_pool.tile([P, QH], F32, tag="s", name=f"s{i}")
                for n in range(2):
                    q0 = qh * QH + n * 512
                    nc.tensor.matmul(
                        s_ps[:, n * 512 : (n + 1) * 512],
                        KT[:, h, kt * P : (kt + 1) * P],
                        QT[:, h, q0 : q0 + 512],
                        start=True, stop=True,
                    )
                pt = pt_pool.tile([P, QH], BF, tag="pt", name=f"pt{i}")
                nc.scalar.activation(pt, s_ps, EXP, bias=0.0, scale=0.125)
                pts[i] = pt

            def pv(i):
                h, qh = phases[i // NS]
                kt = i % NS
                if kt == 0:
                    ots[(h, qh)] = ot_pool.tile([65, QH], F32, tag="ot", name=f"ot{h}_{qh}")
                ot = ots[(h, qh)]
                pt = pts.pop(i)
                for n in range(2):
                    nc.tensor.matmul(
                        ot[:, n * 512 : (n + 1) * 512],
                        V[:, kt, h * 65 : (h + 1) * 65],
                        pt[:, n * 512 : (n + 1) * 512],
                        start=(kt == 0), stop=(kt == NS - 1),
                    )

            def normalize(h, qh, split=False):
                ot = ots.pop((h, qh))
                # the final phase normalizes in two 512 halves so the first
                # tail wo tiles unlock before the full chain finishes
                widths = (512, 512) if split else (QH,)
                c0 = 0
                for w in widths:
                    csl = slice(c0, c0 + w)
                    osb = nrm_pool.tile([DK, QH], F32, tag="osb", name="osb")
                    nc.vector.tensor_copy(out=osb[:, 0:w], in_=ot[0:DK, csl])
                    den = nrm_pool.tile([1, QH], F32, tag="den", name="den")
                    nc.scalar.copy(den[:, 0:w], ot[64:65, csl])
                    recip = nrm_pool.tile([1, QH], F32, tag="recip", name="recip")
                    nc.vector.reciprocal_approx_fast(recip[:, 0:w], den[:, 0:w])
                    rbc = nrm_pool.tile([DK, QH], F32, tag="rbc", name="rbc")
                    nc.gpsimd.partition_broadcast(rbc[:, 0:w], recip[:, 0:w])
                    sl = slice(qh * QH + c0, qh * QH + c0 + w)
                    dst = OC1[0:DK, sl] if h == 0 else (OC1[DK:P, sl] if h == 1 else OC2[0:DK, sl])
                    nc.vector.tensor_mul(dst, osb[:, 0:w], rbc[:, 0:w])
                    c0 += w

            y_r = y.rearrange("(n p) m -> n p m", p=P)

            def writeback(dst, y_ps, st, tail=False):
                y_sb = ysb_pool.tile([P, D], BF, tag="ysb", name=f"y_sb{st}")
                # in the tail the exp stream is done, so ScalarE is free:
                # alternate the PSUM->SBUF cast across vector/scalar so the
                # eight final writebacks drain at 2x
                if tail and st % 2 == 1:
                    nc.scalar.copy(y_sb, y_ps)
                else:
                    nc.vector.tensor_copy(out=y_sb, in_=y_ps)
                nc.sync.dma_start(out=dst, in_=y_sb)

            def wo_parts(y_ps, st, parts):
                sl = slice(st * P, (st + 1) * P)
                # region-major order: adjacent matmuls never accumulate into
                # the same PSUM region (back-to-back same-bank accumulation
                # serializes the PE)
                last = len(parts) - 1
                for pi, (oc, wob) in enumerate(parts):
                    for n0, nn in ((0, 512), (512, 256)):
                        nc.tensor.matmul(
                            y_ps[:, n0 : n0 + nn], oc[:, sl], wob[:, n0 : n0 + nn],
                            start=(pi == 0), stop=(pi == last),
                        )

            def wo_tile(st, tail=False):
                y_ps = ot_pool.tile([P, D], F32, tag="ot", name=f"y_ps{st}")
                wo_parts(y_ps, st, [(OC1, wo_b1), (OC2, wo_b2)])
                writeback(y_r[st], y_ps, st, tail=tail)

            n_steps = len(phases) * NS
            s_exp(0)
            s_exp(1)
            wo_pending = []
            wo_emitted = 0
            for i in range(n_steps):
                if i + 2 < n_steps:
                    s_exp(i + 2)
                pv(i)
                if wo_pending:
                    # spread q-half-0 wo tiles one per step instead of a
                    # blocking burst
                    wo_tile(wo_pending.pop(0))
                    wo_emitted += 1
                h, qh = phases[i // NS]
                kt = i % NS
                if kt == NS - 1:
                    normalize(h, qh, split=((h, qh) == (HG - 1, 1)))
                    if (h, qh) == (HG - 1, 0):
                        wo_pending = list(range(NS // 2))
            for st in range(wo_emitted, NS):
                wo_tile(st, tail=True)

    nc.compile()
    return nc


def kernel(query, key, value, Wq, bq, Wk, bk, Wv, bv, Wo, bo, **_ignored):
    import ml_dtypes
    from concourse.bass_utils import run_bass_kernel_spmd

    bf16 = ml_dtypes.bfloat16
    query = np.asarray(query, dtype=np.float32)
    key = np.asarray(key, dtype=np.float32)
    value = np.asarray(value, dtype=np.float32)
    Wq = np.asarray(Wq, dtype=np.float32)
    Wk = np.asarray(Wk, dtype=np.float32)
    Wv = np.asarray(Wv, dtype=np.float32)
    Wo = np.asarray(Wo, dtype=np.float32)
    bq = np.asarray(bq, dtype=np.float32)
    bk = np.asarray(bk, dtype=np.float32)
    bv = np.asarray(bv, dtype=np.float32)
    bo = np.asarray(bo, dtype=np.float32)

    use_bias_qkv = bool(np.any(bq) or np.any(bk) or np.any(bv))
    if "nc" not in _CACHE or _CACHE.get("bias") != use_bias_qkv:
        _CACHE["nc"] = _build_nc(use_bias_qkv)
        _CACHE["bias"] = use_bias_qkv
    nc = _CACHE["nc"]

    xT = {b: {} for b in range(B)}
    for b in range(B):
        xT[b]["q"] = np.ascontiguousarray(query[b].T).astype(bf16)
        xT[b]["k"] = np.ascontiguousarray(key[b].T).astype(bf16)
        xT[b]["v"] = np.ascontiguousarray(value[b].T).astype(bf16)

    in_maps = []
    for c in range(8):
        b, g = divmod(c, 4)
        hs = slice(g * HD, (g + 1) * HD)
        in_maps.append({
            "xqT": xT[b]["q"],
            "xkT": xT[b]["k"],
            "xvT": xT[b]["v"],
            "wq": np.ascontiguousarray(Wq[:, hs]).astype(bf16),
            "wk": np.ascontiguousarray(Wk[:, hs]).astype(bf16),
            "wv": np.ascontiguousarray(Wv[:, hs]).astype(bf16),
            "wo": np.ascontiguousarray(Wo[hs, :]).astype(bf16),
            "bqkv": np.ascontiguousarray(
                np.stack([bq[hs], bk[hs], bv[hs]]).astype(np.float32)
            ),
        })

    res = run_bass_kernel_spmd(nc, in_maps, core_ids=list(range(8)), **_CACHE.get("run_kwargs", {}))
    _CACHE["last_result"] = res

    out = np.empty((B, S, D), dtype=np.float32)
    for b in range(B):
        acc = res.results[4 * b]["y"].astype(np.float32)
        for g in range(1, 4):
            acc = acc + res.results[4 * b + g]["y"].astype(np.float32)
        out[b] = acc + bo[None, :]
    return out


# revision 34
# speedup vs baseline: 1.1635x; 1.0225x over previous
"""Multi-head attention (B=2, S=2048, D=768, H=12) on 8 Trainium2 cores.

Sharding: core c -> batch b = c // 4, head-group g = c % 4 (3 heads of 12).
Host prep: x^T per batch pre-transposed AND cast to bf16 (halves the x DMA
vs fp32+casting-DMA); weight shards cast to bf16.  Each core projects
Q/K/V for its 3 heads, runs attention, emits its Wo row-shard partial as
bf16; the host sums 4 partials per batch in fp32 and adds bo.

Device kernel (per core):
  - Q^T/K^T stored zero-PADDED to 128 contraction rows ([128, 3, S] tiles,
    rows 64-127 = 0) so every scores matmul is a full 128x128-array
    instruction: the 64-row (half-array / HAM k=4) config measured ~2x
    slower sustained on HW (activity throttle), and padding costs no extra
    PE cycles (row count = rhs free size).
  - All matmul outputs are <=512 fp32 columns (one PSUM bank; 1024-col out
    is an ISA violation, probed).
  - Attention runs as 6 phases (qh-major: (h0,h1,h2) x qh0 then qh1), each
    16 kt steps of: scores 2mm -> exp (ScalarE, [128,1024] tiles) -> PV
    2mm accumulating [65,1024] (ones column in V_aug rides the softmax
    denominator).  A global 2-step software pipeline (scores of step i+2
    emitted before PV of step i) keeps the in-order PE queue from ever
    waiting on the ACT exp, across phase boundaries too.
  - Wo tiles for the first q-half are emitted right after (h2,qh0)'s
    normalize, shortening the serial tail to normalize + 8 wo tiles.
"""

import sys

for _p in ("/opt/trn_rl_repo",):
    if _p not in sys.path:
        sys.path.append(_p)

import numpy as np

B = 2
S = 2048
D = 768
H = 12
DK = 64
HG = 3            # heads per core
HD = HG * DK      # 192
P = 128
NS = S // P       # 16 k-tiles
ND = D // P       # 6 d-chunks
QH = 1024         # q half

_CACHE = {}


def _build_nc(use_bias_qkv):
    import concourse.bacc as bacc
    import concourse.tile as tile
    from concourse import mybir
    from contextlib import ExitStack

    BF = mybir.dt.bfloat16
    F32 = mybir.dt.float32
    EXP = mybir.ActivationFunctionType.Exp

    nc = bacc.Bacc("TRN2", target_bir_lowering=False, debug=False)

    xqT = nc.dram_tensor("xqT", [D, S], BF, kind="ExternalInput").ap()
    xkT = nc.dram_tensor("xkT", [D, S], BF, kind="ExternalInput").ap()
    xvT = nc.dram_tensor("xvT", [D, S], BF, kind="ExternalInput").ap()
    wq = nc.dram_tensor("wq", [D, HD], BF, kind="ExternalInput").ap()
    wk = nc.dram_tensor("wk", [D, HD], BF, kind="ExternalInput").ap()
    wv = nc.dram_tensor("wv", [D, HD], BF, kind="ExternalInput").ap()
    wo = nc.dram_tensor("wo", [HD, D], BF, kind="ExternalInput").ap()
    bqkv = nc.dram_tensor("bqkv", [3, HD], F32, kind="ExternalInput").ap()
    y = nc.dram_tensor("y", [S, D], BF, kind="ExternalOutput").ap()

    with tile.TileContext(nc) as tc, ExitStack() as ctx:
        wpool = ctx.enter_context(tc.tile_pool(name="weights", bufs=1))
        apool = ctx.enter_context(tc.tile_pool(name="acts", bufs=1))

        # zero-padded transposed activations: [:, h, :] = head h, rows 64+ = 0
        KT = apool.tile([P, HG, S], BF, tag="kt")
        QT = apool.tile([P, HG, S], BF, tag="qt")
        V = apool.tile([P, NS, 3 * 65], BF, tag="v")
        OC1 = apool.tile([P, S], BF, tag="oc1")    # heads 0,1 of O^T (normalized)
        OC2 = apool.tile([P, S], BF, tag="oc2")    # head 2, rows 64-127 = 0 (keeps
                                                   # the wo matmuls in full-array config)

        # x chunk tiles (bf16 straight from HBM), all resident
        # x chunks DMA'd in s-halves, first halves of all d-chunks first, so
        # the sbp0 projections can start ~4us after the tensor's DMA begins
        xt_pool = ctx.enter_context(tc.tile_pool(name="xt", bufs=1))
        xtc = {}
        for name, xT in (("wk", xkT), ("wq", xqT), ("wv", xvT)):
            for dc in range(ND):
                xtc[(name, dc)] = xt_pool.tile(
                    [P, S], BF, tag=f"xt_{name}{dc}", name=f"xt_{name}{dc}"
                )
            for half in range(2):
                hsl = slice(half * QH, (half + 1) * QH)
                for dc in range(ND):
                    nc.gpsimd.dma_start(
                        out=xtc[(name, dc)][:, hsl],
                        in_=xT[dc * P : (dc + 1) * P, hsl],
                    )

        # weights (bf16 on host, no device casts), HWDGE queue
        w_bf = {}
        for name, w in (("wk", wk), ("wq", wq), ("wv", wv)):
            wb = wpool.tile([P, ND, HD], BF, tag=f"{name}_bf", name=f"{name}_bf")
            nc.sync.dma_start(out=wb, in_=w.rearrange("(nd p) h -> p nd h", p=P))
            w_bf[name] = wb
        wo_b1 = wpool.tile([P, D], BF, tag="wo_b1")
        nc.sync.dma_start(out=wo_b1, in_=wo[0:P, :])
        wo_b2 = wpool.tile([P, D], BF, tag="wo_b2")   # rows 64-127 = 0 (padding)
        nc.sync.dma_start(out=wo_b2[0:DK, :], in_=wo[P:HD, :])

        bias_a = {}
        bias_b = {}
        bias_vrow = None
        if use_bias_qkv:
            for i, name in enumerate(("wq", "wk", "wv")):
                ba = wpool.tile([P, 1], F32, tag=f"ba_{name}", name=f"ba_{name}")
                nc.sync.dma_start(out=ba, in_=bqkv[i, 0:P].rearrange("p -> p 1"))
                bb = wpool.tile([DK, 1], F32, tag=f"bb_{name}", name=f"bb_{name}")
                nc.sync.dma_start(out=bb, in_=bqkv[i, P:HD].rearrange("p -> p 1"))
                bias_a[name] = ba
                bias_b[name] = bb
            # V bias varies along the free dim of psV [s, 192]: broadcast the
            # bias row across all 128 partitions once
            bvr = wpool.tile([1, HD], F32, tag="bv_row")
            nc.sync.dma_start(out=bvr, in_=bqkv[2, :].rearrange("h -> 1 h"))
            bias_vrow = wpool.tile([P, HD], F32, tag="bv_bcast")
            nc.gpsimd.partition_broadcast(bias_vrow, bvr)

        # padding zeros + V ones columns (off the PE path; after DMA triggers)
        nc.gpsimd.memset(KT[DK:P, :, :], 0.0)
        nc.vector.memset(QT[DK:P, :, :], 0.0)
        nc.vector.memset(V[:, :, 64 : 3 * 65 : 65], 1.0)
        nc.gpsimd.memset(OC2[DK:P, :], 0.0)
        nc.vector.memset(wo_b2[DK:P, :], 0.0)

        # ================= phase 1: projections =================
        with tc.tile_pool(name="ppa", bufs=2, space="PSUM") as ppa_pool, \
             tc.tile_pool(name="ppb", bufs=1, space="PSUM") as ppb_pool, \
             tc.tile_pool(name="psv", bufs=2, space="PSUM") as psv_pool:

            def qk_proj(name, dst):
                wb = w_bf[name]
                for sbp in range(2):
                    sl = slice(sbp * QH, (sbp + 1) * QH)
                    psA = ppa_pool.tile([P, QH], F32, tag="ppa", name=f"psA_{name}{sbp}")
                    psB = ppb_pool.tile([DK, QH], F32, tag="ppb", name=f"psB_{name}{sbp}")
                    for d in range(ND):
                        xt_d = xtc[(name, d)]
                        for half in range(2):
                            hsl = slice(half * 512, (half + 1) * 512)
                            xsl = slice(sbp * QH + half * 512, sbp * QH + (half + 1) * 512)
                            nc.tensor.matmul(
                                psA[:, hsl], wb[:, d, 0:P], xt_d[:, xsl],
                                start=(d == 0), stop=(d == ND - 1),
                            )
                    for d in range(ND):
                        xt_d = xtc[(name, d)]
                        for half in range(2):
                            hsl = slice(half * 512, (half + 1) * 512)
                            xsl = slice(sbp * QH + half * 512, sbp * QH + (half + 1) * 512)
                            nc.tensor.matmul(
                                psB[:, hsl], wb[:, d, P:HD], xt_d[:, xsl],
                                start=(d == 0), stop=(d == ND - 1),
                            )
                    if use_bias_qkv:
                        ba, bb = bias_a[name], bias_b[name]
                        nc.vector.tensor_scalar_add(dst[0:DK, 0, sl], psA[0:DK, :], ba[0:DK])
                        nc.vector.tensor_scalar_add(dst[0:DK, 1, sl], psA[DK:P, :], ba[DK:P])
                        nc.vector.tensor_scalar_add(dst[0:DK, 2, sl], psB, bb)
                    else:
                        nc.vector.tensor_copy(out=dst[0:DK, 0, sl], in_=psA[0:DK, :])
                        nc.vector.tensor_copy(out=dst[0:DK, 1, sl], in_=psA[DK:P, :])
                        nc.vector.tensor_copy(out=dst[0:DK, 2, sl], in_=psB)

            qk_proj("wk", KT)
            qk_proj("wq", QT)

            wb = w_bf["wv"]
            for st in range(NS):
                psV = psv_pool.tile([P, HD], F32, tag="psv", name=f"psV{st}")
                for d in range(ND):
                    nc.tensor.matmul(
                        psV, xtc[("wv", d)][:, st * P : (st + 1) * P], wb[:, d, :],
                        start=(d == 0), stop=(d == ND - 1),
                    )
                for h in range(HG):
                    if use_bias_qkv:
                        nc.vector.tensor_add(
                            V[:, st, h * 65 : h * 65 + 64],
                            psV[:, h * DK : (h + 1) * DK],
                            bias_vrow[:, h * DK : (h + 1) * DK],
                        )
                    else:
                        nc.vector.tensor_copy(
                            out=V[:, st, h * 65 : h * 65 + 64],
                            in_=psV[:, h * DK : (h + 1) * DK],
                        )

        # ============ phase 2: attention (+ wo) ============
        with tc.tile_pool(name="s_ps", bufs=2, space="PSUM") as s_pool, \
             tc.tile_pool(name="ot_ps", bufs=2, space="PSUM") as ot_pool, \
             tc.tile_pool(name="pt", bufs=6) as pt_pool, \
             tc.tile_pool(name="nrm", bufs=2) as nrm_pool, \
             tc.tile_pool(name="y_sb", bufs=2) as ysb_pool:

            # qh-major so wo for q-half 0 can run before the final phase
            phases = [(h, 0) for h in range(HG)] + [(h, 1) for h in range(HG)]
            ots = {}
            pts = {}

            def s_exp(i):
                h, qh = phases[i // NS]
                kt = i % NS
                s_ps = s_pool.tile([P, QH], F32, tag="s", name=f"s{i}")
                for n in range(2):
                    q0 = qh * QH + n * 512
                    nc.tensor.matmul(
                        s_ps[:, n * 512 : (n + 1) * 512],
                        KT[:, h, kt * P : (kt + 1) * P],
                        QT[:, h, q0 : q0 + 512],
                        start=True, stop=True,
                    )
                pt = pt_pool.tile([P, QH], BF, tag="pt", name=f"pt{i}")
                nc.scalar.activation(pt, s_ps, EXP, bias=0.0, scale=0.125)
                pts[i] = pt

            def pv(i):
                h, qh = phases[i // NS]
                kt = i % NS
                if kt == 0:
                    ots[(h, qh)] = ot_pool.tile([65, QH], F32, tag="ot", name=f"ot{h}_{qh}")
                ot = ots[(h, qh)]
                pt = pts.pop(i)
                for n in range(2):
                    nc.tensor.matmul(
                        ot[:, n * 512 : (n + 1) * 512],
                        V[:, kt, h * 65 : (h + 1) * 65],
                        pt[:, n * 512 : (n + 1) * 512],
                        start=(kt == 0), stop=(kt == NS - 1),
                    )

            def normalize(h, qh, split=False):
                ot = ots.pop((h, qh))
                # the final phase normalizes in two 512 halves so the first
                # tail wo tiles unlock before the full chain finishes
                widths = (512, 512) if split else (QH,)
                c0 = 0
                for w in widths:
                    csl = slice(c0, c0 + w)
                    osb = nrm_pool.tile([DK, QH], F32, tag="osb", name="osb")
                    nc.vector.tensor_copy(out=osb[:, 0:w], in_=ot[0:DK, csl])
                    # den copy on vector, not scalar: a scalar-queue copy at a
                    # phase boundary delays the next phase's exps by ~1us
                    den = nrm_pool.tile([1, QH], F32, tag="den", name="den")
                    nc.vector.tensor_copy(out=den[:, 0:w], in_=ot[64:65, csl])
                    recip = nrm_pool.tile([1, QH], F32, tag="recip", name="recip")
                    nc.vector.reciprocal_approx_fast(recip[:, 0:w], den[:, 0:w])
                    rbc = nrm_pool.tile([DK, QH], F32, tag="rbc", name="rbc")
                    nc.gpsimd.partition_broadcast(rbc[:, 0:w], recip[:, 0:w])
                    sl = slice(qh * QH + c0, qh * QH + c0 + w)
                    dst = OC1[0:DK, sl] if h == 0 else (OC1[DK:P, sl] if h == 1 else OC2[0:DK, sl])
                    nc.vector.tensor_mul(dst, osb[:, 0:w], rbc[:, 0:w])
                    c0 += w

            y_r = y.rearrange("(n p) m -> n p m", p=P)

            def writeback(dst, y_ps, st, tail=False):
                y_sb = ysb_pool.tile([P, D], BF, tag="ysb", name=f"y_sb{st}")
                # in the tail the exp stream is done, so ScalarE is free:
                # alternate the PSUM->SBUF cast across vector/scalar so the
                # eight final writebacks drain at 2x
                if tail and st % 2 == 1:
                    nc.scalar.copy(y_sb, y_ps)
                else:
                    nc.vector.tensor_copy(out=y_sb, in_=y_ps)
                nc.sync.dma_start(out=dst, in_=y_sb)

            def wo_parts(y_ps, st, parts):
                sl = slice(st * P, (st + 1) * P)
                # region-major order: adjacent matmuls never accumulate into
                # the same PSUM region (back-to-back same-bank accumulation
                # serializes the PE)
                last = len(parts) - 1
                for pi, (oc, wob) in enumerate(parts):
                    for n0, nn in ((0, 512), (512, 256)):
                        nc.tensor.matmul(
                            y_ps[:, n0 : n0 + nn], oc[:, sl], wob[:, n0 : n0 + nn],
                            start=(pi == 0), stop=(pi == last),
                        )

            def wo_tile(st, tail=False):
                y_ps = ot_pool.tile([P, D], F32, tag="ot", name=f"y_ps{st}")
                wo_parts(y_ps, st, [(OC1, wo_b1), (OC2, wo_b2)])
                writeback(y_r[st], y_ps, st, tail=tail)

            n_steps = len(phases) * NS
            s_exp(0)
            s_exp(1)
            wo_pending = []
            wo_emitted = 0
            for i in range(n_steps):
                if i + 2 < n_steps:
                    s_exp(i + 2)
                pv(i)
                if wo_pending:
                    # spread q-half-0 wo tiles one per step instead of a
                    # blocking burst
                    wo_tile(wo_pending.pop(0))
                    wo_emitted += 1
                h, qh = phases[i // NS]
                kt = i % NS
                if kt == NS - 1:
                    normalize(h, qh, split=((h, qh) == (HG - 1, 1)))
                    if (h, qh) == (HG - 1, 0):
                        wo_pending = list(range(NS // 2))
            for st in range(wo_emitted, NS):
                wo_tile(st, tail=True)

    nc.compile()
    return nc


def kernel(query, key, value, Wq, bq, Wk, bk, Wv, bv, Wo, bo, **_ignored):
    import ml_dtypes
    from concourse.bass_utils import run_bass_kernel_spmd

    bf16 = ml_dtypes.bfloat16
    query = np.asarray(query, dtype=np.float32)
    key = np.asarray(key, dtype=np.float32)
    value = np.asarray(value, dtype=np.float32)
    Wq = np.asarray(Wq, dtype=np.float32)
    Wk = np.asarray(Wk, dtype=np.float32)
    Wv = np.asarray(Wv, dtype=np.float32)
    Wo = np.asarray(Wo, dtype=np.float32)
    bq = np.asarray(bq, dtype=np.float32)
    bk = np.asarray(bk, dtype=np.float32)
    bv = np.asarray(bv, dtype=np.float32)
    bo = np.asarray(bo, dtype=np.float32)

    use_bias_qkv = bool(np.any(bq) or np.any(bk) or np.any(bv))
    if "nc" not in _CACHE or _CACHE.get("bias") != use_bias_qkv:
        _CACHE["nc"] = _build_nc(use_bias_qkv)
        _CACHE["bias"] = use_bias_qkv
    nc = _CACHE["nc"]

    xT = {b: {} for b in range(B)}
    for b in range(B):
        xT[b]["q"] = np.ascontiguousarray(query[b].T).astype(bf16)
        xT[b]["k"] = np.ascontiguousarray(key[b].T).astype(bf16)
        xT[b]["v"] = np.ascontiguousarray(value[b].T).astype(bf16)

    in_maps = []
    for c in range(8):
        b, g = divmod(c, 4)
        hs = slice(g * HD, (g + 1) * HD)
        in_maps.append({
            "xqT": xT[b]["q"],
            "xkT": xT[b]["k"],
            "xvT": xT[b]["v"],
            "wq": np.ascontiguousarray(Wq[:, hs]).astype(bf16),
            "wk": np.ascontiguousarray(Wk[:, hs]).astype(bf16),
            "wv": np.ascontiguousarray(Wv[:, hs]).astype(bf16),
            "wo": np.ascontiguousarray(Wo[hs, :]).astype(bf16),
            "bqkv": np.ascontiguousarray(
                np.stack([bq[hs], bk[hs], bv[hs]]).astype(np.float32)
            ),
        })

    res = run_bass_kernel_spmd(nc, in_maps, core_ids=list(range(8)), **_CACHE.get("run_kwargs", {}))
    _CACHE["last_result"] = res

    out = np.empty((B, S, D), dtype=np.float32)
    for b in range(B):
        acc = res.results[4 * b]["y"].astype(np.float32)
        for g in range(1, 4):
            acc = acc + res.results[4 * b + g]["y"].astype(np.float32)
        out[b] = acc + bo[None, :]
    return out
